# revision 15
# baseline (speedup 1.0000x reference)
"""Trainium2 Bass kernel for nn_GaussianSampler.

Computation (per reference):
    stats = x @ W.T + b          # [B,S,2N]
    mu    = stats[..., N:]
    var   = softplus(stats[..., :N])
    std_mat = diag_embed(var)    # [B,S,N,N], only diagonal nonzero
    sample  = mu + sqrt(var) * eps

Sharding: pure data parallel over the B*S = 256 tokens -> 32 tokens per core
across 8 cores; the Linear weights are replicated.

Key optimization: ExternalOutput buffers are pre-zeroed by the runtime
(run_bass_kernel_spmd native path zeroes out_maps; the axon/PJRT path donates
zero buffers - see concourse/bass2jax.py), so the kernel only writes the
diagonal of std_mat via a strided scatter DMA instead of materializing
32 MiB of zeros per core.
"""

import os
from contextlib import ExitStack

import numpy as np

import concourse.bass as bass
import concourse.mybir as mybir
import concourse.tile as tile
from concourse import bacc
from concourse.bass_utils import run_bass_kernel_spmd

N = 512          # feature dim
M = 2 * N        # linear output dim
NCORES = 8
T = 256          # total tokens (B*S = 2*128)
TPC = T // NCORES  # tokens per core = 32
KC = N // 128    # contraction chunks = 4

F32 = mybir.dt.float32

# Kept for test.py introspection (exec_time_ns etc).
LAST_RESULT = None


def _build_program():
    nc = bacc.Bacc(
        "TRN2",
        target_bir_lowering=False,
        debug=False,
        enable_asserts=False,
        num_devices=NCORES,
    )

    # Split weight load so the var-half matmuls + softplus chain overlap the
    # mu-half load: wa = concat([x^T, Wvar^T], axis=1), wb = Wmu^T.
    WAW = TPC + N
    wa = nc.dram_tensor("wa", [N, WAW], F32, kind="ExternalInput")
    wb = nc.dram_tensor("wb", [N, N], F32, kind="ExternalInput")
    bb = nc.dram_tensor("bb", [1, M], F32, kind="ExternalInput")
    ep = nc.dram_tensor("ep", [TPC, N], F32, kind="ExternalInput")

    samp = nc.dram_tensor("samp", [TPC, N], F32, kind="ExternalOutput")
    muo = nc.dram_tensor("muo", [TPC, N], F32, kind="ExternalOutput")
    stdm = nc.dram_tensor("stdm", [TPC, N, N], F32, kind="ExternalOutput")

    with tile.TileContext(nc) as tc, ExitStack() as ctx:
        pool = ctx.enter_context(tc.tile_pool(name="sb", bufs=1))
        psum = ctx.enter_context(tc.tile_pool(name="ps", bufs=1, space="PSUM"))

        # ---- loads ----
        # [x^T | Wvar^T] first (gates the var half), Wmu^T second
        wa_all = pool.tile([128, KC, WAW], F32, tag="wa")
        for j in range(2):
            nc.sync.dma_start(
                out=wa_all[:, 2 * j:2 * j + 2, :],
                in_=bass.AP(wa, j * 256 * WAW,
                            [[WAW, 128], [128 * WAW, 2], [1, WAW]]),
            )
        wb_all = pool.tile([128, KC, N], F32, tag="wb")
        for j in range(2):
            nc.sync.dma_start(
                out=wb_all[:, 2 * j:2 * j + 2, :],
                in_=bass.AP(wb, j * 256 * N,
                            [[N, 128], [128 * N, 2], [1, N]]),
            )
        xt_t = [wa_all[:, kc, 0:TPC] for kc in range(KC)]
        wv_t = [wa_all[:, kc, TPC:TPC + N] for kc in range(KC)]
        wm_t = [wb_all[:, kc, :] for kc in range(KC)]
        ep_t = pool.tile([TPC, N], F32, tag="ep")
        nc.sync.dma_start(out=ep_t, in_=ep[:, :])
        # bias broadcast across the TPC partitions
        bb_t = pool.tile([TPC, M], F32, tag="bb")
        nc.sync.dma_start(out=bb_t, in_=bass.AP(bb, 0, [[0, TPC], [1, M]]))

        # ---- matmul: stats[t, m] = sum_n xT[n, t] * wt[n, m] ----
        ps_var = psum.tile([TPC, N], F32, tag="psv")
        ps_mu = psum.tile([TPC, N], F32, tag="psm")
        for kc in range(KC):
            nc.tensor.matmul(
                ps_var[:, :], xt_t[kc], wv_t[kc],
                start=(kc == 0), stop=(kc == KC - 1),
            )
        for kc in range(KC):
            nc.tensor.matmul(
                ps_mu[:, :], xt_t[kc], wm_t[kc],
                start=(kc == 0), stop=(kc == KC - 1),
            )

        # ---- bias add + activations ----
        pre = pool.tile([TPC, N], F32, tag="pre")
        nc.vector.tensor_add(pre[:, :], ps_var[:, :], bb_t[:, 0:N])
        mu_t = pool.tile([TPC, N], F32, tag="mu")
        nc.vector.tensor_add(mu_t[:, :], ps_mu[:, :], bb_t[:, N:M])

        # softplus(x) = ln(exp(x) + 1); |stats| is O(5) so exp cannot overflow
        ex_t = pool.tile([TPC, N], F32, tag="ex")
        nc.scalar.activation(ex_t[:, :], pre[:, :],
                             mybir.ActivationFunctionType.Exp)
        var_t = pool.tile([TPC, N], F32, tag="var")
        nc.scalar.activation(var_t[:, :], ex_t[:, :],
                             mybir.ActivationFunctionType.Ln, bias=1.0)
        std_t = pool.tile([TPC, N], F32, tag="std")
        nc.scalar.activation(std_t[:, :], var_t[:, :],
                             mybir.ActivationFunctionType.Sqrt)

        prod = pool.tile([TPC, N], F32, tag="prod")
        nc.vector.tensor_mul(prod[:, :], std_t[:, :], ep_t[:, :])
        samp_t = pool.tile([TPC, N], F32, tag="samp")
        nc.vector.tensor_add(samp_t[:, :], prod[:, :], mu_t[:, :])

        # ---- outputs ----
        nc.sync.dma_start(out=samp[:, :], in_=samp_t[:, :])
        nc.sync.dma_start(out=muo[:, :], in_=mu_t[:, :])
        # diagonal scatter: stdm[t, i, i] = var[t, i]
        diag_dst = bass.AP(stdm, 0, [[N * N, TPC], [N + 1, N], [1, 1]])
        nc.scalar.dma_start(out=diag_dst, in_=var_t[:, :])

    nc.compile()
    return nc


_NC = None


def _get_nc():
    global _NC
    if _NC is None:
        _NC = _build_program()
    return _NC


def kernel(x, W, b, eps):
    global LAST_RESULT
    x = np.ascontiguousarray(np.asarray(x, np.float32).reshape(T, N))
    eps = np.ascontiguousarray(np.asarray(eps, np.float32).reshape(T, N))
    W = np.asarray(W, np.float32)
    b = np.asarray(b, np.float32)

    wv_host = W[:N].T                            # [N, N] var-param cols
    wb_host = np.ascontiguousarray(W[N:].T)      # [N, N] mu-param cols
    bb_host = np.ascontiguousarray(b.reshape(1, M))

    in_maps = []
    for i in range(NCORES):
        xs = x[TPC * i:TPC * (i + 1)]            # [TPC, N]
        in_maps.append({
            "wa": np.ascontiguousarray(
                np.concatenate([xs.T, wv_host], axis=1)),  # [N, TPC+N]
            "wb": wb_host,
            "bb": bb_host,
            "ep": np.ascontiguousarray(eps[TPC * i:TPC * (i + 1)]),
        })

    nc = _get_nc()
    LAST_RESULT = run_bass_kernel_spmd(
        nc, in_maps, core_ids=list(range(NCORES)),
        trace=bool(int(os.environ.get("KERNEL_TRACE", "0"))),
    )
    res = LAST_RESULT.results

    sample = np.concatenate([r["samp"] for r in res], 0).reshape(2, 128, N)
    mu = np.concatenate([r["muo"] for r in res], 0).reshape(2, 128, N)
    std_mat = np.concatenate([r["stdm"] for r in res], 0).reshape(2, 128, N, N)
    return sample, mu, std_mat
